# revision 1
# baseline (speedup 1.0000x reference)
"""Trainium2 Bass kernel for nn_Conv_39273180955616.

Computes, for X:(16,64,512,512) f32, K:(1,1,7,7), b:(1,1,1,1):
    out[n,c] = correlate2d(X[n,c], Keff, pad=3) + 49*b
where Keff = K.sum(axis=(0,1)).

Strategy: pure data parallel over the 1024 (n,c) planes -> 128 planes/core
on 8 cores.  Per plane, the 7x7 correlation runs on TensorE as
banded-Toeplitz matmuls: the h-dimension contraction is a [K<=128, 128]
band matrix (7 diagonals of one kernel column) against an image block
(rows on partitions), and the 7 w-shifts are free-dim offsets into a
zero-padded (W+6) image row, accumulated in PSUM.  The 24-row bottom
tiles of 4 consecutive planes are packed into one block-diagonal matmul
set (stacked on partitions), cutting the matmul count by 15%.  Inputs
are pre-cast to bf16 on host (PSUM accumulates in fp32); bias is added
during PSUM->SBUF eviction, alternating ScalarE/VectorE.  DMA is
batched and spread across the SP-HWDGE and SWDGE rings.
"""
import numpy as np
import ml_dtypes

import concourse.bass as bass
import concourse.tile as tile
from concourse import bacc, mybir
from concourse.bass_utils import run_bass_kernel_spmd

N_CORES = 8
H = 512
W = 512
WPAD = W + 6  # 3 zero columns each side
N_PLANES_TOTAL = 16 * 64
PLANES_PER_CORE = N_PLANES_TOTAL // N_CORES  # 128
GROUP = 4  # planes per bottom-tile merge group

# Per-plane tiles: 4 x 122 output rows (kinds 0/1); the 24-row bottom
# tile (kind 2) is handled once per GROUP planes as a block-diagonal
# [108, 96] matmul (4 x K=27 / M=24 blocks stacked on partitions).
# (out_row0, out_rows, in_row0, in_rows, kind)
TILES = [
    (0, 122, 0, 125, 0),
    (122, 122, 119, 128, 1),
    (244, 122, 241, 128, 1),
    (366, 122, 363, 128, 1),
]
KIND_K = {0: 125, 1: 128, 2: GROUP * 27}
M_PAD = 128  # lhsT padded to 128 cols -> FWL eligible; pad rows are zero
WCOLS = 3 * 7 * M_PAD


def _build_weight_pack(Keff: np.ndarray) -> np.ndarray:
    """Keff (7,7) f32 -> packed banded-Toeplitz lhsT matrices [128, WCOLS] bf16.

    Matrix for (kind, dw) sits at cols [(kind*7+dw)*128, ...+128).
    lhsT[p, m] = Keff[dh, dw], dh = p - m (+3 for kind 0); matmul computes
    out[m, w] = sum_p lhsT[p, m] * block[p, w + dw].  Kind 2 is the
    block-diagonal stack of GROUP bottom tiles: block g at rows
    [27g, 27g+27) x cols [24g, 24g+24).
    """
    wp = np.zeros((128, WCOLS), np.float32)
    for kind in (0, 1):
        Kk = KIND_K[kind]
        p = np.arange(Kk)[:, None]
        m = np.arange(122)[None, :]
        dh = p - m + (3 if kind == 0 else 0)
        ok = (dh >= 0) & (dh < 7)
        for dw in range(7):
            mat = np.zeros((Kk, M_PAD), np.float32)
            mat[:, :122][ok] = Keff[dh[ok], dw]
            c0 = (kind * 7 + dw) * M_PAD
            wp[:Kk, c0:c0 + M_PAD] = mat
    # kind 2 block-diagonal
    p = np.arange(27)[:, None]
    m = np.arange(24)[None, :]
    dh = p - m
    ok = (dh >= 0) & (dh < 7)
    for dw in range(7):
        blk = np.zeros((27, 24), np.float32)
        blk[ok] = Keff[dh[ok], dw]
        c0 = (2 * 7 + dw) * M_PAD
        for g in range(GROUP):
            wp[27 * g:27 * g + 27, c0 + 24 * g:c0 + 24 * g + 24] = blk
    return wp.astype(ml_dtypes.bfloat16)


_NC_CACHE = {}


def _get_module(n_planes: int):
    if n_planes in _NC_CACHE:
        return _NC_CACHE[n_planes]
    assert n_planes % GROUP == 0
    nc = bacc.Bacc("TRN2", target_bir_lowering=False, debug=False,
                   num_devices=N_CORES)
    xp = nc.dram_tensor("xp", [n_planes, H, WPAD], mybir.dt.bfloat16,
                        kind="ExternalInput")
    wt = nc.dram_tensor("wt", [128, WCOLS], mybir.dt.bfloat16,
                        kind="ExternalInput")
    bv = nc.dram_tensor("bv", [128, 1], mybir.dt.float32,
                        kind="ExternalInput")
    out = nc.dram_tensor("out", [n_planes, H, W], mybir.dt.float32,
                         kind="ExternalOutput")

    x_elems = H * WPAD  # per-plane element count in xp

    with tile.TileContext(nc) as tc:
        with (
            tc.tile_pool(name="wp", bufs=1) as wpool,
            tc.tile_pool(name="xa", bufs=8) as xapool,
            tc.tile_pool(name="xb", bufs=8) as xbpool,
            tc.tile_pool(name="xg", bufs=3) as xgpool,
            tc.tile_pool(name="ps", bufs=8, space="PSUM") as pspool,
            tc.tile_pool(name="ob", bufs=10) as obpool,
            tc.tile_pool(name="og", bufs=3) as ogpool,
        ):
            wtile = wpool.tile([128, WCOLS], mybir.dt.bfloat16)
            nc.sync.dma_start(wtile[:], wt.ap())
            btile = wpool.tile([128, 1], mybir.dt.float32)
            nc.sync.dma_start(btile[:], bv.ap())

            def evict(engine, dst, src, rows):
                if engine == "act":
                    nc.scalar.activation(
                        dst, src, mybir.ActivationFunctionType.Identity,
                        bias=btile[:rows, :], scale=1.0)
                else:
                    nc.vector.tensor_scalar_add(dst, src, btile[:rows, :])

            for g0 in range(0, n_planes, GROUP):
                # bottom rows (485..511) of GROUP planes in one load
                xg = xgpool.tile([GROUP * 27, WPAD], mybir.dt.bfloat16)
                for g in range(GROUP):
                    nc.sync.dma_start(
                        xg[27 * g:27 * g + 27, :],
                        bass.AP(xp, (g0 + g) * x_elems + 485 * WPAD,
                                [[WPAD, 27], [1, WPAD]]))
                for p in range(g0, g0 + GROUP):
                    # ---- input loads (SP ring) ----
                    xa = xapool.tile([125, WPAD], mybir.dt.bfloat16)
                    nc.sync.dma_start(
                        xa[:], bass.AP(xp, p * x_elems,
                                       [[WPAD, 125], [1, WPAD]]))
                    xb = xbpool.tile([128, 3 * WPAD], mybir.dt.bfloat16)
                    # rows 119+122b+q, b=0..2 (overlapping strided read)
                    nc.sync.dma_start(
                        xb[:].rearrange("p (b w) -> p b w", b=3),
                        bass.AP(xp, p * x_elems + 119 * WPAD,
                                [[WPAD, 128], [122 * WPAD, 3], [1, WPAD]]))

                    ob = obpool.tile([122, 4 * W], mybir.dt.float32)
                    for t, (or0, oh, ir0, ih, kind) in enumerate(TILES):
                        if kind == 0:
                            rhs_of = lambda dw: xa[:, dw:dw + W]
                        else:
                            b = t - 1
                            rhs_of = lambda dw, b=b: xb[:, b * WPAD + dw:
                                                        b * WPAD + dw + W]
                        pt = pspool.tile([128, W], mybir.dt.float32)
                        for dw in range(7):
                            c0 = (kind * 7 + dw) * M_PAD
                            nc.tensor.matmul(
                                pt[:, :], wtile[:ih, c0:c0 + M_PAD],
                                rhs_of(dw), start=(dw == 0), stop=(dw == 6))
                        evict("act" if t % 2 == 0 else "dve",
                              ob[:, t * W:(t + 1) * W], pt[:122, :], 122)
                    # rows 0..487 = 4 tiles of 122 (1 MB); alternate the
                    # SWDGE and ACT-HWDGE rings so store completions keep up
                    store_eng = nc.gpsimd if p % 2 == 0 else nc.scalar
                    store_eng.dma_start(
                        bass.AP(out, p * H * W,
                                [[W, 122], [122 * W, 4], [1, W]]),
                        ob[:].rearrange("p (b w) -> p b w", b=4))

                # ---- merged bottom tiles of the group ----
                pt = pspool.tile([128, W], mybir.dt.float32)
                for dw in range(7):
                    c0 = (2 * 7 + dw) * M_PAD
                    nc.tensor.matmul(
                        pt[:, :], wtile[:GROUP * 27, c0:c0 + M_PAD],
                        xg[:, dw:dw + W], start=(dw == 0), stop=(dw == 6))
                og = ogpool.tile([GROUP * 24, W], mybir.dt.float32)
                evict("act", og[:], pt[:GROUP * 24, :], GROUP * 24)
                for g in range(GROUP):
                    nc.gpsimd.dma_start(
                        bass.AP(out, ((g0 + g) * H + 488) * W,
                                [[W, 24], [1, W]]),
                        og[24 * g:24 * g + 24, :])

    nc.compile()
    _NC_CACHE[n_planes] = nc
    return nc


def _prep_inputs(X, K, b, n_cores=N_CORES):
    Keff = np.asarray(K, np.float32).sum(axis=(0, 1))
    wt = _build_weight_pack(Keff)
    bias = np.float32(np.asarray(b).reshape(-1)[0]) * np.float32(K.size)
    bv = np.full((128, 1), bias, np.float32)

    Xr = np.asarray(X, np.float32).reshape(-1, H, W)
    n_total = Xr.shape[0]
    per = n_total // n_cores
    Xp = np.zeros((n_total, H, WPAD), ml_dtypes.bfloat16)
    Xp[:, :, 3:3 + W] = Xr.astype(ml_dtypes.bfloat16)
    in_maps = [
        {"xp": Xp[i * per:(i + 1) * per], "wt": wt, "bv": bv}
        for i in range(n_cores)
    ]
    return in_maps, per


def kernel(X, K, b):
    in_maps, per = _prep_inputs(X, K, b)
    nc = _get_module(per)
    res = run_bass_kernel_spmd(nc, in_maps, list(range(N_CORES)))
    out = np.concatenate([res.results[i]["out"] for i in range(N_CORES)], axis=0)
    return out.reshape(np.asarray(X).shape)



# revision 6
# speedup vs baseline: 1.5331x; 1.5331x over previous
"""Trainium2 Bass kernel for nn_Conv_39273180955616.

Computes, for X:(16,64,512,512) f32, K:(1,1,7,7), b:(1,1,1,1):
    out[n,c] = correlate2d(X[n,c], Keff, pad=3) + 49*b
where Keff = K.sum(axis=(0,1)).

Strategy: pure data parallel over the 1024 (n,c) planes -> 128 planes/core
on 8 cores.  Per plane, the 7x7 correlation runs on TensorE as
banded-Toeplitz matmuls: the h-dimension contraction is a [K<=128, 128]
band matrix (7 diagonals of one kernel column) against an image block
(rows on partitions), and the 7 w-shifts are free-dim offsets into a
zero-padded (W+6) image row, accumulated in PSUM.  The 24-row bottom
tiles of 4 consecutive planes are packed into one block-diagonal matmul
set (stacked on partitions), cutting the matmul count by 15%.  Inputs
are pre-cast to bf16 on host (PSUM accumulates in fp32); bias is added
during PSUM->SBUF eviction, alternating ScalarE/VectorE.  DMA is
batched and spread across the SP-HWDGE and SWDGE rings.
"""
import numpy as np
import ml_dtypes

import concourse.bass as bass
import concourse.tile as tile
from concourse import bacc, mybir
from concourse.bass_utils import run_bass_kernel_spmd

N_CORES = 8
H = 512
W = 512
WPAD = W + 6  # 3 zero columns each side
N_PLANES_TOTAL = 16 * 64
PLANES_PER_CORE = N_PLANES_TOTAL // N_CORES  # 128
GROUP = 4  # planes per bottom-tile merge group

# Per-plane tiles: 4 x 122 output rows (kinds 0/1); the 24-row bottom
# tile (kind 2) is handled once per GROUP planes as a block-diagonal
# [108, 96] matmul (4 x K=27 / M=24 blocks stacked on partitions).
# (out_row0, out_rows, in_row0, in_rows, kind)
TILES = [
    (0, 122, 0, 125, 0),
    (122, 122, 119, 128, 1),
    (244, 122, 241, 128, 1),
    (366, 122, 363, 128, 1),
]
KIND_K = {0: 125, 1: 128, 2: GROUP * 27}
M_PAD = 128  # lhsT padded to 128 cols -> FWL eligible; pad rows are zero
WCOLS = 3 * 7 * M_PAD


def _build_weight_pack(Keff: np.ndarray) -> np.ndarray:
    """Keff (7,7) f32 -> packed banded-Toeplitz lhsT matrices [128, WCOLS] bf16.

    Matrix for (kind, dw) sits at cols [(kind*7+dw)*128, ...+128).
    lhsT[p, m] = Keff[dh, dw], dh = p - m (+3 for kind 0); matmul computes
    out[m, w] = sum_p lhsT[p, m] * block[p, w + dw].  Kind 2 is the
    block-diagonal stack of GROUP bottom tiles: block g at rows
    [27g, 27g+27) x cols [24g, 24g+24).
    """
    wp = np.zeros((128, WCOLS), np.float32)
    for kind in (0, 1):
        Kk = KIND_K[kind]
        p = np.arange(Kk)[:, None]
        m = np.arange(122)[None, :]
        dh = p - m + (3 if kind == 0 else 0)
        ok = (dh >= 0) & (dh < 7)
        for dw in range(7):
            mat = np.zeros((Kk, M_PAD), np.float32)
            mat[:, :122][ok] = Keff[dh[ok], dw]
            c0 = (kind * 7 + dw) * M_PAD
            wp[:Kk, c0:c0 + M_PAD] = mat
    # kind 2 block-diagonal
    p = np.arange(27)[:, None]
    m = np.arange(24)[None, :]
    dh = p - m
    ok = (dh >= 0) & (dh < 7)
    for dw in range(7):
        blk = np.zeros((27, 24), np.float32)
        blk[ok] = Keff[dh[ok], dw]
        c0 = (2 * 7 + dw) * M_PAD
        for g in range(GROUP):
            wp[27 * g:27 * g + 27, c0 + 24 * g:c0 + 24 * g + 24] = blk
    return wp.astype(ml_dtypes.bfloat16)


_NC_CACHE = {}


def _get_module(n_planes: int):
    if n_planes in _NC_CACHE:
        return _NC_CACHE[n_planes]
    assert n_planes % GROUP == 0
    nc = bacc.Bacc("TRN2", target_bir_lowering=False, debug=False,
                   num_devices=N_CORES)
    xp = nc.dram_tensor("xp", [n_planes, H, WPAD], mybir.dt.bfloat16,
                        kind="ExternalInput")
    wt = nc.dram_tensor("wt", [128, WCOLS], mybir.dt.bfloat16,
                        kind="ExternalInput")
    bv = nc.dram_tensor("bv", [128, 1], mybir.dt.float32,
                        kind="ExternalInput")
    out = nc.dram_tensor("out", [n_planes, H, W], mybir.dt.bfloat16,
                         kind="ExternalOutput")

    x_elems = H * WPAD  # per-plane element count in xp

    with tile.TileContext(nc) as tc:
        with (
            tc.tile_pool(name="wp", bufs=1) as wpool,
            tc.tile_pool(name="xa", bufs=8) as xapool,
            tc.tile_pool(name="xb", bufs=8) as xbpool,
            tc.tile_pool(name="xg", bufs=3) as xgpool,
            tc.tile_pool(name="ps", bufs=8, space="PSUM") as pspool,
            tc.tile_pool(name="ob", bufs=10) as obpool,
            tc.tile_pool(name="og", bufs=3) as ogpool,
        ):
            wtile = wpool.tile([128, WCOLS], mybir.dt.bfloat16)
            nc.sync.dma_start(wtile[:], wt.ap())
            btile = wpool.tile([128, 1], mybir.dt.float32)
            nc.sync.dma_start(btile[:], bv.ap())

            def evict(engine, dst, src, rows):
                nc.vector.tensor_scalar_add(dst, src, btile[:rows, :])

            for g0 in range(0, n_planes, GROUP):
                # bottom rows (485..511) of GROUP planes in one load
                xg = xgpool.tile([GROUP * 27, WPAD], mybir.dt.bfloat16)
                for g in range(GROUP):
                    nc.sync.dma_start(
                        xg[27 * g:27 * g + 27, :],
                        bass.AP(xp, (g0 + g) * x_elems + 485 * WPAD,
                                [[WPAD, 27], [1, WPAD]]))
                for p in range(g0, g0 + GROUP):
                    # ---- input loads (SP ring) ----
                    xa = xapool.tile([125, WPAD], mybir.dt.bfloat16)
                    nc.sync.dma_start(
                        xa[:], bass.AP(xp, p * x_elems,
                                       [[WPAD, 125], [1, WPAD]]))
                    xb = xbpool.tile([128, 3 * WPAD], mybir.dt.bfloat16)
                    # rows 119+122b+q, b=0..2 (overlapping strided read)
                    nc.sync.dma_start(
                        xb[:].rearrange("p (b w) -> p b w", b=3),
                        bass.AP(xp, p * x_elems + 119 * WPAD,
                                [[WPAD, 128], [122 * WPAD, 3], [1, WPAD]]))

                    ob = obpool.tile([122, 4 * W], mybir.dt.bfloat16)
                    for t, (or0, oh, ir0, ih, kind) in enumerate(TILES):
                        if kind == 0:
                            rhs_of = lambda dw: xa[:, dw:dw + W]
                        else:
                            b = t - 1
                            rhs_of = lambda dw, b=b: xb[:, b * WPAD + dw:
                                                        b * WPAD + dw + W]
                        pt = pspool.tile([128, W], mybir.dt.float32)
                        for dw in range(7):
                            c0 = (kind * 7 + dw) * M_PAD
                            nc.tensor.matmul(
                                pt[:, :], wtile[:ih, c0:c0 + M_PAD],
                                rhs_of(dw), start=(dw == 0), stop=(dw == 6))
                        evict("act" if t % 2 == 0 else "dve",
                              ob[:, t * W:(t + 1) * W], pt[:122, :], 122)
                    # rows 0..487 = 4 tiles of 122 (1 MB); alternate the
                    # SWDGE and ACT-HWDGE rings so store completions keep up
                    store_eng = nc.gpsimd if p % 2 == 0 else nc.scalar
                    store_eng.dma_start(
                        bass.AP(out, p * H * W,
                                [[W, 122], [122 * W, 4], [1, W]]),
                        ob[:].rearrange("p (b w) -> p b w", b=4))

                # ---- merged bottom tiles of the group ----
                pt = pspool.tile([128, W], mybir.dt.float32)
                for dw in range(7):
                    c0 = (2 * 7 + dw) * M_PAD
                    nc.tensor.matmul(
                        pt[:, :], wtile[:GROUP * 27, c0:c0 + M_PAD],
                        xg[:, dw:dw + W], start=(dw == 0), stop=(dw == 6))
                og = ogpool.tile([GROUP * 24, W], mybir.dt.bfloat16)
                evict("act", og[:], pt[:GROUP * 24, :], GROUP * 24)
                for g in range(GROUP):
                    nc.gpsimd.dma_start(
                        bass.AP(out, ((g0 + g) * H + 488) * W,
                                [[W, 24], [1, W]]),
                        og[24 * g:24 * g + 24, :])

    nc.compile()
    _NC_CACHE[n_planes] = nc
    return nc


def _prep_inputs(X, K, b, n_cores=N_CORES):
    Keff = np.asarray(K, np.float32).sum(axis=(0, 1))
    wt = _build_weight_pack(Keff)
    bias = np.float32(np.asarray(b).reshape(-1)[0]) * np.float32(K.size)
    bv = np.full((128, 1), bias, np.float32)

    Xr = np.asarray(X, np.float32).reshape(-1, H, W)
    n_total = Xr.shape[0]
    per = n_total // n_cores
    Xp = np.zeros((n_total, H, WPAD), ml_dtypes.bfloat16)
    Xp[:, :, 3:3 + W] = Xr.astype(ml_dtypes.bfloat16)
    in_maps = [
        {"xp": Xp[i * per:(i + 1) * per], "wt": wt, "bv": bv}
        for i in range(n_cores)
    ]
    return in_maps, per


def kernel(X, K, b):
    in_maps, per = _prep_inputs(X, K, b)
    nc = _get_module(per)
    res = run_bass_kernel_spmd(nc, in_maps, list(range(N_CORES)))
    out = np.concatenate(
        [np.asarray(res.results[i]["out"], np.float32) for i in range(N_CORES)],
        axis=0)
    return out.reshape(np.asarray(X).shape)



# revision 8
# speedup vs baseline: 1.7254x; 1.1254x over previous
"""Trainium2 Bass kernel for nn_Conv_39273180955616.

Computes, for X:(16,64,512,512) f32, K:(1,1,7,7), b:(1,1,1,1):
    out[n,c] = correlate2d(X[n,c], Keff, pad=3) + 49*b
where Keff = K.sum(axis=(0,1)).

Data parallel over the 1024 (n,c) planes -> 128 planes/core on 8 cores.

Per core the image planes are stored h-major in HBM (Xt[h, plane, w],
zero-padded to 518 in h and w) so one DMA descriptor carries a 16-plane
row run (16.6 KB).  The 7x7 correlation runs on TensorE as banded-
Toeplitz matmuls packed 4-per-pass onto the 128x128 PE array via 64x64
tile_position tiles: row half r in {0,64} holds a 64-row h-window (58
output rows), col half c in {0,64} computes a different plane, and the
7 kernel-column shifts accumulate in PSUM as free-dim offsets into the
518-wide padded rows.  Windows chain in pairs across the per-core
(block, window) list; the last window of each plane overlaps rows with
the previous one (band cols m<10 zeroed) so every window is full-size.
PSUM is evicted to SBUF as bf16 by DVE/ACT copies; stores go out over
the gpsimd SWDGE ring with 2 KB descriptors ([h, plane, w] bf16 layout,
plane pairs packed per partition).  The +49*b bias and the f32 cast are
applied on the host during the gather.
"""
import numpy as np
import ml_dtypes

import concourse.bass as bass
import concourse.tile as tile
from concourse import bacc, mybir
from concourse.bass_utils import run_bass_kernel_spmd

N_CORES = 8
H = 512
W = 512
HPAD = H + 6
WPAD = W + 6
PLANES = 16 * 64 // N_CORES     # 128 planes per core
BLK = 16                        # planes per block (descriptor run length)
NBLK = PLANES // BLK            # 8
WIN = 64                        # input rows per window
WOUT = 58                       # output rows per window (WIN - 6)
NWIN = 9                        # windows per plane: 8 @ stride 58 + 1 tail
TAIL_BASE = HPAD - WIN          # padded row base of tail window = 454
TAIL_MLO = 10                   # tail window valid outputs: m in [10, 58)


def _win_base(t):
    return 58 * t if t < 8 else TAIL_BASE


def _build_weights(Keff: np.ndarray) -> np.ndarray:
    """wb [128, 2*7*64] bf16: variant v (0 normal, 1 tail), dw in 0..6.

    B[p, m] = Keff[p-m, dw] for 0 <= p-m <= 6, m in [mlo, 58), else 0.
    Both 64-partition halves hold the same content.
    """
    wb = np.zeros((128, 2 * 7 * 64), np.float32)
    p = np.arange(64)[:, None]
    m = np.arange(64)[None, :]
    dh = p - m
    for v in range(2):
        mlo = TAIL_MLO if v == 1 else 0
        ok = (dh >= 0) & (dh < 7) & (m >= mlo) & (m < WOUT)
        for dw in range(7):
            blk = np.zeros((64, 64), np.float32)
            blk[ok] = Keff[dh[ok], dw]
            c0 = (v * 7 + dw) * 64
            wb[0:64, c0:c0 + 64] = blk
            wb[64:128, c0:c0 + 64] = blk
    return wb.astype(ml_dtypes.bfloat16)


_NC_CACHE = {}


def _get_module():
    if "nc" in _NC_CACHE:
        return _NC_CACHE["nc"]
    nc = bacc.Bacc("TRN2", target_bir_lowering=False, debug=False,
                   num_devices=N_CORES)
    xt = nc.dram_tensor("xt", [HPAD, PLANES, WPAD], mybir.dt.bfloat16,
                        kind="ExternalInput")
    wt = nc.dram_tensor("wt", [128, 2 * 7 * 64], mybir.dt.bfloat16,
                        kind="ExternalInput")
    out = nc.dram_tensor("out", [H, PLANES, W], mybir.dt.bfloat16,
                         kind="ExternalOutput")

    # flat (block, window) list; consecutive entries pair onto the two
    # 64-partition halves of one macrotile
    wis = [(b, t) for b in range(NBLK) for t in range(NWIN)]
    assert len(wis) % 2 == 0

    with tile.TileContext(nc) as tc:
        with (
            tc.tile_pool(name="wp", bufs=1) as wpool,
            tc.tile_pool(name="mt", bufs=3) as mtpool,
            tc.tile_pool(name="ps", bufs=3, space="PSUM") as pspool,
            tc.tile_pool(name="ev", bufs=4) as evpool,
        ):
            wb = wpool.tile([128, 2 * 7 * 64], mybir.dt.bfloat16)
            nc.sync.dma_start(wb[:], wt.ap())

            for pair in range(len(wis) // 2):
                (bA, tA), (bB, tB) = wis[2 * pair], wis[2 * pair + 1]
                mt = mtpool.tile([128, BLK * WPAD], mybir.dt.bfloat16)
                for half, (b, t) in enumerate(((bA, tA), (bB, tB))):
                    nc.sync.dma_start(
                        mt[64 * half:64 * half + 64, :],
                        bass.AP(xt,
                                _win_base(t) * PLANES * WPAD + BLK * b * WPAD,
                                [[PLANES * WPAD, WIN], [1, BLK * WPAD]]))

                for j in range(4):          # store-pair of spans
                    evs = [evpool.tile([128, 1024], mybir.dt.bfloat16,
                                       name=f"ev{h}") for h in range(2)]
                    for par in range(2):    # span parity within the pair
                        k = 2 * j + par
                        pts = [pspool.tile([128, W], mybir.dt.float32,
                                           name=f"pt{h}") for h in range(2)]
                        for dw in range(7):
                            for half, t in enumerate((tA, tB)):
                                r = 64 * half
                                v = 1 if t == 8 else 0
                                for c in (0, 64):
                                    pl = k + 8 * (c // 64)
                                    nc.tensor.matmul(
                                        pts[half][c:c + 64, :],
                                        wb[r:r + 64,
                                           (v * 7 + dw) * 64:(v * 7 + dw + 1) * 64],
                                        mt[r:r + 64, pl * WPAD + dw:
                                           pl * WPAD + dw + W],
                                        start=(dw == 0), stop=(dw == 6),
                                        tile_position=(r, c))
                        # evict both banks: fp32 PSUM -> bf16 SBUF copies
                        for half in range(2):
                            dst = evs[half][:, par * 512:par * 512 + 512]
                            if (half + par) % 2 == 0:
                                nc.vector.tensor_copy(dst, pts[half][:, :])
                            else:
                                nc.scalar.copy(dst, pts[half][:, :])
                    # stores: 2 per bank (one per 64-partition half of the
                    # PSUM layout), 2KB descriptors; out row = win_base + m
                    for half, (b, t) in enumerate(((bA, tA), (bB, tB))):
                        mlo = TAIL_MLO if t == 8 else 0
                        nm = WOUT - mlo
                        row0 = _win_base(t) + mlo
                        for q in range(2):
                            eng = nc.gpsimd if half == 0 else nc.sync
                            eng.dma_start(
                                bass.AP(out,
                                        (row0 * PLANES + BLK * b + 2 * j
                                         + 8 * q) * W,
                                        [[PLANES * W, nm], [1, 2 * W]]),
                                evs[half][64 * q + mlo:64 * q + mlo + nm, :])

    nc.compile()
    _NC_CACHE["nc"] = nc
    return nc


def _prep_inputs(X, K, b, n_cores=N_CORES):
    Keff = np.asarray(K, np.float32).sum(axis=(0, 1))
    wt = _build_weights(Keff)
    Xr = np.asarray(X, np.float32).reshape(-1, H, W)
    X16 = Xr.astype(ml_dtypes.bfloat16)
    in_maps = []
    for i in range(n_cores):
        Xt = np.zeros((HPAD, PLANES, WPAD), ml_dtypes.bfloat16)
        Xt[3:3 + H, :, 3:3 + W] = X16[i * PLANES:(i + 1) * PLANES].transpose(1, 0, 2)
        in_maps.append({"xt": Xt, "wt": wt})
    bias = np.float32(np.asarray(b, np.float32).reshape(-1)[0]) * np.float32(
        np.asarray(K).size)
    return in_maps, bias


def kernel(X, K, b):
    in_maps, bias = _prep_inputs(X, K, b)
    nc = _get_module()
    res = run_bass_kernel_spmd(nc, in_maps, list(range(N_CORES)))
    shape = np.asarray(X).shape
    out = np.empty((N_CORES * PLANES, H, W), np.float32)
    for i in range(N_CORES):
        # res out: [H, PLANES, W] bf16 -> [PLANES, H, W] f32 + bias
        oc = np.asarray(res.results[i]["out"])
        out[i * PLANES:(i + 1) * PLANES] = (
            oc.transpose(1, 0, 2).astype(np.float32) + bias)
    return out.reshape(shape)


# revision 10
# speedup vs baseline: 2.5277x; 1.4650x over previous
"""Trainium2 Bass kernel for nn_Conv_39273180955616.

Computes, for X:(16,64,512,512) f32, K:(1,1,7,7), b:(1,1,1,1):
    out[n,c] = correlate2d(X[n,c], Keff, pad=3) + 49*b
where Keff = K.sum(axis=(0,1)).

Data parallel over the 1024 (n,c) planes -> 128 planes/core on 8 cores.

Per core the image planes are stored h-major in HBM (Xt[h, plane, w],
zero-padded to 518 in h and w) so one DMA descriptor carries a 16-plane
row run (16.6 KB).  The 7x7 correlation runs on TensorE as banded-
Toeplitz matmuls packed 4-per-pass onto the 128x128 PE array via 64x64
tile_position tiles: row half r in {0,64} holds a 64-row h-window (58
output rows), col half c in {0,64} computes a different plane, and the
7 kernel-column shifts accumulate in PSUM as free-dim offsets into the
518-wide padded rows.  Windows chain in pairs across the per-core
(block, window) list; the last window of each plane overlaps rows with
the previous one (band cols m<10 zeroed) so every window is full-size.
PSUM is evicted to SBUF as bf16 by DVE/ACT copies; stores go out over
the gpsimd SWDGE ring with 2 KB descriptors ([h, plane, w] bf16 layout,
plane pairs packed per partition).  The +49*b bias and the f32 cast are
applied on the host during the gather.
"""
import numpy as np
import ml_dtypes

import concourse.bass as bass
import concourse.tile as tile
from concourse import bacc, mybir
from concourse.bass_utils import run_bass_kernel_spmd

N_CORES = 8
H = 512
W = 512
HPAD = H + 6
WPAD = W + 6
PLANES = 16 * 64 // N_CORES     # 128 planes per core
BLK = 16                        # planes per block (descriptor run length)
NBLK = PLANES // BLK            # 8
WIN = 64                        # input rows per window
WOUT = 58                       # output rows per window (WIN - 6)
NWIN = 9                        # windows per plane: 8 @ stride 58 + 1 tail
TAIL_BASE = HPAD - WIN          # padded row base of tail window = 454
TAIL_MLO = 10                   # tail window valid outputs: m in [10, 58)


def _win_base(t):
    return 58 * t if t < 8 else TAIL_BASE


def _build_weights(Keff: np.ndarray) -> np.ndarray:
    """wb [128, 2*7*64] bf16: variant v (0 normal, 1 tail), dw in 0..6.

    B[p, m] = Keff[p-m, dw] for 0 <= p-m <= 6, m in [mlo, 58), else 0.
    Both 64-partition halves hold the same content.
    """
    wb = np.zeros((128, 2 * 7 * 64), np.float32)
    p = np.arange(64)[:, None]
    m = np.arange(64)[None, :]
    dh = p - m
    for v in range(2):
        mlo = TAIL_MLO if v == 1 else 0
        ok = (dh >= 0) & (dh < 7) & (m >= mlo) & (m < WOUT)
        for dw in range(7):
            blk = np.zeros((64, 64), np.float32)
            blk[ok] = Keff[dh[ok], dw]
            c0 = (v * 7 + dw) * 64
            wb[0:64, c0:c0 + 64] = blk
            wb[64:128, c0:c0 + 64] = blk
    return wb.astype(ml_dtypes.bfloat16)


_NC_CACHE = {}


def _get_module():
    if "nc" in _NC_CACHE:
        return _NC_CACHE["nc"]
    nc = bacc.Bacc("TRN2", target_bir_lowering=False, debug=False,
                   num_devices=N_CORES)
    xt = nc.dram_tensor("xt", [HPAD, PLANES, WPAD], mybir.dt.bfloat16,
                        kind="ExternalInput")
    wt = nc.dram_tensor("wt", [128, 2 * 7 * 64], mybir.dt.bfloat16,
                        kind="ExternalInput")
    out = nc.dram_tensor("out", [H, PLANES, W], mybir.dt.bfloat16,
                         kind="ExternalOutput")

    # flat (block, window) list; consecutive entries pair onto the two
    # 64-partition halves of one macrotile
    wis = [(b, t) for b in range(NBLK) for t in range(NWIN)]
    assert len(wis) % 2 == 0

    with tile.TileContext(nc) as tc:
        with (
            tc.tile_pool(name="wp", bufs=1) as wpool,
            tc.tile_pool(name="mt", bufs=3) as mtpool,
            tc.tile_pool(name="ps", bufs=3, space="PSUM") as pspool,
            tc.tile_pool(name="ev", bufs=4) as evpool,
        ):
            wb = wpool.tile([128, 2 * 7 * 64], mybir.dt.bfloat16)
            nc.sync.dma_start(wb[:], wt.ap())

            for pair in range(len(wis) // 2):
                (bA, tA), (bB, tB) = wis[2 * pair], wis[2 * pair + 1]
                mt = mtpool.tile([128, BLK * WPAD], mybir.dt.bfloat16)
                for half, (b, t) in enumerate(((bA, tA), (bB, tB))):
                    nc.sync.dma_start(
                        mt[64 * half:64 * half + 64, :],
                        bass.AP(xt,
                                _win_base(t) * PLANES * WPAD + BLK * b * WPAD,
                                [[PLANES * WPAD, WIN], [1, BLK * WPAD]]))

                for j in range(2):          # store-group of 4 spans
                    evs = [evpool.tile([128, 2048], mybir.dt.bfloat16,
                                       name=f"ev{h}") for h in range(2)]
                    for par in range(4):    # span index within the group
                        k = 4 * j + par
                        pts = [pspool.tile([128, W], mybir.dt.float32,
                                           name=f"pt{h}") for h in range(2)]
                        for dw in range(7):
                            for half, t in enumerate((tA, tB)):
                                r = 64 * half
                                v = 1 if t == 8 else 0
                                for c in (0, 64):
                                    pl = k + 8 * (c // 64)
                                    nc.tensor.matmul(
                                        pts[half][c:c + 64, :],
                                        wb[r:r + 64,
                                           (v * 7 + dw) * 64:(v * 7 + dw + 1) * 64],
                                        mt[r:r + 64, pl * WPAD + dw:
                                           pl * WPAD + dw + W],
                                        start=(dw == 0), stop=(dw == 6),
                                        tile_position=(r, c))
                        # evict both banks: fp32 PSUM -> bf16 SBUF copies
                        for half in range(2):
                            dst = evs[half][:, par * 512:par * 512 + 512]
                            if (half + par) % 2 == 0:
                                nc.vector.tensor_copy(dst, pts[half][:, :])
                            else:
                                nc.scalar.copy(dst, pts[half][:, :])
                    # stores (SWDGE, async): 2 per bank (one per 64-partition
                    # half of the PSUM layout), 4KB descriptors
                    for half, (b, t) in enumerate(((bA, tA), (bB, tB))):
                        mlo = TAIL_MLO if t == 8 else 0
                        nm = WOUT - mlo
                        row0 = _win_base(t) + mlo
                        for q in range(2):
                            nc.gpsimd.dma_start(
                                bass.AP(out,
                                        (row0 * PLANES + BLK * b + 4 * j
                                         + 8 * q) * W,
                                        [[PLANES * W, nm], [1, 4 * W]]),
                                evs[half][64 * q + mlo:64 * q + mlo + nm, :])

    nc.compile()
    _NC_CACHE["nc"] = nc
    return nc


def _prep_inputs(X, K, b, n_cores=N_CORES):
    Keff = np.asarray(K, np.float32).sum(axis=(0, 1))
    wt = _build_weights(Keff)
    Xr = np.asarray(X, np.float32).reshape(-1, H, W)
    X16 = Xr.astype(ml_dtypes.bfloat16)
    in_maps = []
    for i in range(n_cores):
        Xt = np.zeros((HPAD, PLANES, WPAD), ml_dtypes.bfloat16)
        Xt[3:3 + H, :, 3:3 + W] = X16[i * PLANES:(i + 1) * PLANES].transpose(1, 0, 2)
        in_maps.append({"xt": Xt, "wt": wt})
    bias = np.float32(np.asarray(b, np.float32).reshape(-1)[0]) * np.float32(
        np.asarray(K).size)
    return in_maps, bias


def kernel(X, K, b):
    in_maps, bias = _prep_inputs(X, K, b)
    nc = _get_module()
    res = run_bass_kernel_spmd(nc, in_maps, list(range(N_CORES)))
    shape = np.asarray(X).shape
    out = np.empty((N_CORES * PLANES, H, W), np.float32)
    for i in range(N_CORES):
        # res out: [H, PLANES, W] bf16 -> [PLANES, H, W] f32 + bias
        oc = np.asarray(res.results[i]["out"])
        out[i * PLANES:(i + 1) * PLANES] = (
            oc.transpose(1, 0, 2).astype(np.float32) + bias)
    return out.reshape(shape)
